# revision 61
# baseline (speedup 1.0000x reference)
"""Batch Graph-Attention layer (GAT, eval mode) on 8 Trainium2 NeuronCores.

Math per graph b (reference):
    Wh = h @ W                         (N=1024, Fo=64)
    f1 = Wh @ a1 ; f2 = Wh @ a2        (N,)
    e[i,j]   = leakyrelu(f1[i]+f2[j], 0.01)
    att      = softmax(e, axis=j)
    out      = elu(att @ Wh)

Device algorithm (per graph), avoiding any O(N^2) transcendentals and
using only ONE O(N^2) element-wise op per 128-row chunk:
    exp(lrelu(x)) == max(exp(x), exp(0.01x))          (exact for slope in (0,1))
    expe[i,j] = max(g1[i]g2[j], hh1[i]hh2[j])          g=exp(f), hh=exp(0.01 f)
              = g1[i] * u[j,i],   u[j,i] = max(g2[j], hh2[j]*r1[i])
    with r1[i] = exp(-0.99 f1[i]).  The g1[i] factor cancels in the
    softmax ratio, so it is never computed:
    u_c   = (r1b * hh2_c) max g2_c      (ONE fused DVE tensor_scalar)
    numer.T[o,i] & rowsum[i] via PE:  lhsT = [Wh | 1] (65 cols), rhs = u
    out[i,:] = elu(numer[i,:]/rowsum[i]),
    elu(x) = max(x, min(exp(x)-1, 0))   (exact identity)

Sharding: batch dim 16 -> 8 cores x 2 graphs (pure data parallel).
Node ids use the (p c) permutation -- partition p holds rows 8p..8p+7 --
so every h/out DMA run is 1KB+ contiguous per partition; softmax sums
are permutation-invariant and the store applies the inverse permutation.
Emission is phase-interleaved (A0 A1 B0 B1 C0 C1) so the two graphs
pipeline across engines.
"""

import numpy as np

import concourse.bass as bass
import concourse.mybir as mybir
import concourse.tile as tile
from concourse import bacc

F32 = mybir.dt.float32
BF16 = mybir.dt.bfloat16
AF = mybir.ActivationFunctionType
OP = mybir.AluOpType

B_PER_CORE = 2
N = 1024
F_IN = 128
F_OUT = 64
C = N // 128  # 8 chunks of 128 rows
NEG_SLOPE = 0.01
WARM = 8

LAST_PERF = {}


def build_bass():
    nc = bacc.Bacc("TRN2", target_bir_lowering=False, debug=False)

    h_d = nc.dram_tensor("h", [B_PER_CORE, N, F_IN], F32, kind="ExternalInput")
    w_d = nc.dram_tensor("W", [B_PER_CORE, F_IN, F_OUT], F32, kind="ExternalInput")
    a_d = nc.dram_tensor("a", [B_PER_CORE, 1, 2 * F_OUT, 1], F32, kind="ExternalInput")
    o_d = nc.dram_tensor("out", [B_PER_CORE, N, F_OUT], F32, kind="ExternalOutput")

    with tile.TileContext(nc) as tc:
        with (
            tc.tile_pool(name="singles", bufs=1) as singles,
            tc.tile_pool(name="hin", bufs=2) as hin_pool,
            tc.tile_pool(name="ht", bufs=2) as ht_pool,
            tc.tile_pool(name="small", bufs=2) as small_pool,
            tc.tile_pool(name="bcast", bufs=2) as bcast_pool,
            tc.tile_pool(name="v", bufs=16) as v_pool,
            tc.tile_pool(name="tail", bufs=2) as tail_pool,
            tc.tile_pool(name="ps", bufs=7, space="PSUM") as ps,
            tc.tile_pool(name="pswarm", bufs=1, space="PSUM") as pswarm,
        ):
            # identity built on-chip (no DMA): 1 where col==row else 0
            ident = singles.tile([128, 128], F32)
            nc.gpsimd.memset(ident[:], 1.0)
            nc.gpsimd.affine_select(
                ident[:], ident[:], pattern=[[1, 128]],
                compare_op=OP.is_equal, fill=0.0,
                base=0, channel_multiplier=-1,
            )
            zeros = singles.tile([128, 128], F32)
            nc.vector.memset(zeros[:], 0.0)

            warm_ps = pswarm.tile([128, 128], F32, tag="warm")

            st = [dict() for _ in range(B_PER_CORE)]

            def stage_load(b):
                """W/a DMAs first on each ring so the wt/wa preamble PE work
                can start as soon as the rings spin up."""
                s = st[b]
                dmaw = nc.scalar if b == 0 else nc.sync
                wext = small_pool.tile([128, F_OUT + 1], F32, tag="wext")
                dmaw.dma_start(out=wext[:, 0:F_OUT], in_=w_d[b])
                apair = small_pool.tile([F_OUT, 2], F32, tag="apair")
                dmaw.dma_start(
                    out=apair[:],
                    in_=a_d[b, 0, :, 0].rearrange("(two o) -> o two", two=2),
                )
                s.update(wext=wext, apair=apair)

            def stage_h(b):
                """One 512B-per-partition piece per chunk, alternating across
                the two DMA rings (fine pieces let the first h-transposes
                start while the rest of h streams in)."""
                s = st[b]
                h_sb = hin_pool.tile([128, C * F_IN], F32, tag="h")
                h_src = h_d[b].rearrange("(p c) f -> p (c f)", c=C)
                w = F_IN
                for q in range(8):
                    eng = (nc.sync, nc.scalar)[q % 2]
                    eng.dma_start(
                        out=h_sb[:, q * w : (q + 1) * w],
                        in_=h_src[:, q * w : (q + 1) * w],
                    )
                s["h_sb"] = h_sb

            def stage_wt(b):
                """W.T, wa12 = W @ [a1|a2], wa1 broadcast, bf16 W-ext.
                Runs in the preamble DMA window (before the h transposes)."""
                s = st[b]
                wext = s["wext"]
                wt_ps = ps.tile([F_OUT, 128], F32, tag="ps")
                nc.tensor.transpose(wt_ps[:], wext[:, 0:F_OUT], ident[:])
                wt_sb = small_pool.tile([F_OUT, 128], F32, tag="wt")
                nc.vector.tensor_copy(wt_sb[:], wt_ps[:])

                wa_ps = ps.tile([128, 2], F32, tag="ps")
                nc.tensor.matmul(wa_ps[:], wt_sb[:], s["apair"][:])
                wa_sb = small_pool.tile([128, 2], F32, tag="wa")
                nc.vector.tensor_copy(wa_sb[:], wa_ps[:])
                # wa2 becomes column 64 of the Wh matmul rhs -> f2 per chunk
                nc.vector.tensor_copy(wext[:, F_OUT : F_OUT + 1], wa_sb[:, 1:2])
                wextb = small_pool.tile([128, F_OUT + 1], BF16, tag="wextb")
                nc.vector.tensor_copy(wextb[:], wext[:])
                # broadcast wa1 along free dim -> lhsT for the f1-broadcast mm
                wa1b = small_pool.tile([128, 128], BF16, tag="wa1b")
                nc.vector.tensor_scalar(
                    wa1b[:], zeros[:], wa_sb[:, 0:1], None, op0=OP.add
                )
                s.update(wextb=wextb, wa1b=wa1b)

            def stage_a(b):
                """h transposes (bf16 ht), Wh+f2, small exps (g2, hh2),
                whb = [Wh|1], f1 broadcast, r1b = exp(-0.99 f1b)."""
                s = st[b]
                # transpose h -> ht [f, n] (bf16: feeds only bf16 matmuls)
                ht_sb = ht_pool.tile([128, N], BF16, tag="ht")
                for half in range(2):
                    pt = ps.tile([128, 4, 128], F32, tag="ps")
                    for q in range(4):
                        c = half * 4 + q
                        nc.tensor.transpose(
                            pt[:, q, :],
                            s["h_sb"][:, c * F_IN : (c + 1) * F_IN],
                            ident[:],
                        )
                    # split each PSUM->SBUF half-copy across Scalar and DVE:
                    # halves the latency on the ht -> pwh/f1b -> r1b chain
                    o = half * 512
                    nc.scalar.copy(ht_sb[:, o : o + 256], pt[:, 0:2, :])
                    nc.vector.tensor_copy(
                        ht_sb[:, o + 256 : o + 512], pt[:, 2:4, :]
                    )

                # Wh chunks (+f2 col): out[:, c, 0:64] = Wh_c, [:, c, 64] = f2_c
                pwh = []
                for half in range(2):
                    p = ps.tile([128, 4, F_OUT + 1], F32, tag="ps")
                    pwh.append(p)
                    for q in range(4):
                        c = half * 4 + q
                        nc.tensor.matmul(
                            p[:, q, :],
                            ht_sb[:, c * 128 : (c + 1) * 128],
                            s["wextb"][:],
                        )

                # B-phase per-partition scalars: g2 = exp(f2), hh2 = exp(.01 f2)
                g2 = small_pool.tile([128, C], F32, tag="g2")
                hh2 = small_pool.tile([128, C], F32, tag="hh2")
                for half in range(2):
                    sl = slice(half * 4, (half + 1) * 4)
                    nc.scalar.activation(g2[:, sl], pwh[half][:, :, F_OUT], AF.Exp)
                for half in range(2):
                    sl = slice(half * 4, (half + 1) * 4)
                    nc.scalar.activation(
                        hh2[:, sl], pwh[half][:, :, F_OUT], AF.Exp, scale=NEG_SLOPE
                    )

                # f1 broadcast to all 128 partitions via PE (bf16 fast path)
                pf1b = []
                for half in range(2):
                    p = ps.tile([128, 512], F32, tag="ps")
                    pf1b.append(p)
                    nc.tensor.matmul(
                        p[:], s["wa1b"][:], ht_sb[:, half * 512 : (half + 1) * 512]
                    )

                # r1b = exp(-0.99 f1b)  [128, 1024] bf16 (B-phase in0)
                r1b = bcast_pool.tile([128, N], BF16, tag="r1b")
                for half in range(2):
                    sl = slice(half * 512, (half + 1) * 512)
                    nc.scalar.activation(
                        r1b[:, sl], pf1b[half][:], AF.Exp,
                        scale=-(1.0 - NEG_SLOPE),
                    )

                # whb = [Wh | 1 | 1]: 66 cols so the per-chunk 64-col bf16
                # writes stay 4-byte aligned; col 64 is the rowsum ones-column.
                whb = small_pool.tile([128, C, F_OUT + 2], BF16, tag="whb")
                nc.vector.memset(whb[:], 1.0)
                for half in range(2):
                    nc.vector.tensor_copy(
                        whb[:, half * 4 : (half + 1) * 4, 0:F_OUT],
                        pwh[half][:, :, 0:F_OUT],
                    )
                s.update(ht=ht_sb, g2=g2, hh2=hh2, whb=whb, r1b=r1b)

            def stage_b(b):
                """Per chunk: u_c = (r1b * hh2_c) max g2_c in ONE fused DVE op,
                then the accumulating final matmuls (numer.T | rowsum)."""
                s = st[b]
                u_tiles = []
                for c in range(C):
                    u = v_pool.tile([128, N], BF16, tag="u")
                    u_tiles.append(u)
                    nc.vector.tensor_scalar(
                        u[:], s["r1b"][:],
                        s["hh2"][:, c : c + 1], s["g2"][:, c : c + 1],
                        op0=OP.mult, op1=OP.max,
                    )

                phpT = []
                for half in range(2):
                    p = ps.tile([F_OUT + 1, 512], F32, tag="ps")
                    phpT.append(p)
                    for c in range(C):
                        nc.tensor.matmul(
                            p[:],
                            s["whb"][:, c, 0 : F_OUT + 1],
                            u_tiles[c][:, half * 512 : (half + 1) * 512],
                            start=(c == 0),
                            stop=(c == C - 1),
                        )
                s["phpT"] = phpT

            def stage_c(b):
                """Transpose numer.T back, normalize, ELU, store."""
                s = st[b]
                hpT_sb = tail_pool.tile([F_OUT + 1, N], F32, tag="hpT")
                for half in range(2):
                    dst = hpT_sb[:, half * 512 : (half + 1) * 512]
                    if half == 0:
                        nc.scalar.copy(dst, s["phpT"][half][:])
                    else:
                        nc.vector.tensor_copy(dst, s["phpT"][half][:])
                php = []
                for half in range(2):
                    p = ps.tile([128, 4, F_OUT + 1], F32, tag="ps")
                    php.append(p)
                    for q in range(4):
                        c = half * 4 + q
                        nc.tensor.transpose(
                            p[:, q, :],
                            hpT_sb[:, c * 128 : (c + 1) * 128],
                            ident[: F_OUT + 1, : F_OUT + 1],
                        )

                rz = small_pool.tile([128, C], F32, tag="rz")
                for half in range(2):
                    sl = slice(half * 4, (half + 1) * 4)
                    nc.vector.reciprocal(rz[:, sl], php[half][:, :, F_OUT])
                # normalize all 4 chunks of a half in ONE tensor_tensor with a
                # 0-stride broadcast of rz along the feature dim (the 8 small
                # per-chunk ops were fixed-overhead dominated)
                hp = tail_pool.tile([128, C, F_OUT], F32, tag="hp")
                for half in range(2):
                    sl = slice(half * 4, (half + 1) * 4)
                    rz3 = rz[:, sl].rearrange("p (c one) -> p c one", one=1)
                    b_in0, b_rz = bass.broadcast_tensor_aps(
                        php[half][:, :, 0:F_OUT], rz3
                    )
                    nc.vector.tensor_tensor(hp[:, sl], b_in0, b_rz, op=OP.mult)
                # elu(x) = max(x, min(exp(x)-1, 0)); per-half so the store DMA
                # of half 0 overlaps the tail compute of half 1.
                te = tail_pool.tile([128, C, F_OUT], F32, tag="te")
                sm = tail_pool.tile([128, C, F_OUT], F32, tag="sm")
                osb = tail_pool.tile([128, C, F_OUT], F32, tag="osb")
                o_dst = o_d[b].rearrange("(p c) o -> p (c o)", c=C)
                osb_flat = osb[:].rearrange("p c o -> p (c o)")
                w = 4 * F_OUT
                for half in range(2):
                    sl = slice(half * 4, (half + 1) * 4)
                    nc.scalar.activation(te[:, sl], hp[:, sl], AF.Exp)
                    nc.vector.tensor_scalar(
                        sm[:, sl], te[:, sl], 1.0, 0.0,
                        op0=OP.subtract, op1=OP.min,
                    )
                    nc.vector.tensor_tensor(
                        osb[:, sl], hp[:, sl], sm[:, sl], op=OP.max
                    )
                    eng = (nc.sync, nc.scalar) if b == 0 else (nc.scalar, nc.sync)
                    eng[half].dma_start(
                        out=o_dst[:, half * w : (half + 1) * w],
                        in_=osb_flat[:, half * w : (half + 1) * w],
                    )

            # Ring issue order == emission order: W/a first (tiny), then the
            # h pieces; consumers are emitted before later transfers on the
            # same ring so their coarse ring-sem waits stay tight.
            stage_load(0)
            stage_load(1)
            stage_wt(0)
            stage_wt(1)
            stage_h(0)
            stage_h(1)
            # PE warm-up: junk matmuls in the DMA window so the HAM clock gate
            # ramps before the real PE work arrives.
            for _ in range(WARM):
                nc.tensor.matmul(warm_ps[:], zeros[:], zeros[:])
            stage_a(0)
            stage_a(1)
            # keep PE busy across the A->B seam (waiting on r1b/u) so the
            # HAM clock gate never ramps back down mid-kernel
            for _ in range(6):
                nc.tensor.matmul(warm_ps[:], zeros[:], zeros[:])
            stage_b(0)
            stage_b(1)
            stage_c(0)
            stage_c(1)

    nc.compile()
    return nc


def kernel(h: np.ndarray, W: np.ndarray, a: np.ndarray, _trace: bool = False):
    from concourse.bass_utils import run_bass_kernel_spmd

    n_cores = 8
    nc = build_bass()
    in_maps = []
    for i in range(n_cores):
        sl = slice(i * B_PER_CORE, (i + 1) * B_PER_CORE)
        in_maps.append(
            {
                "h": np.ascontiguousarray(h[sl]),
                "W": np.ascontiguousarray(W[sl]),
                "a": np.ascontiguousarray(a[sl]),
            }
        )
    res = run_bass_kernel_spmd(
        nc, in_maps, core_ids=list(range(n_cores)), trace=_trace
    )
    LAST_PERF.clear()
    LAST_PERF.update(
        {
            "exec_time_ns": res.exec_time_ns,
            "mean_exec_time_ns": res.mean_exec_time_ns,
            "trace": res.instructions_and_trace[1]
            if res.instructions_and_trace
            else None,
        }
    )
    return np.concatenate([r["out"] for r in res.results], axis=0)


# revision 62
# speedup vs baseline: 1.2072x; 1.2072x over previous
"""Batch Graph-Attention layer (GAT, eval mode) on 8 Trainium2 NeuronCores.

Math per graph b (reference):
    Wh = h @ W                         (N=1024, Fo=64)
    f1 = Wh @ a1 ; f2 = Wh @ a2        (N,)
    e[i,j]   = leakyrelu(f1[i]+f2[j], 0.01)
    att      = softmax(e, axis=j)
    out      = elu(att @ Wh)

Device algorithm (per graph), avoiding any O(N^2) transcendentals and
using only ONE O(N^2) element-wise op per 128-row chunk:
    exp(lrelu(x)) == max(exp(x), exp(0.01x))          (exact for slope in (0,1))
    expe[i,j] = max(g1[i]g2[j], hh1[i]hh2[j])          g=exp(f), hh=exp(0.01 f)
              = g1[i] * u[j,i],   u[j,i] = max(g2[j], hh2[j]*r1[i])
    with r1[i] = exp(-0.99 f1[i]).  The g1[i] factor cancels in the
    softmax ratio, so it is never computed:
    u_c   = (r1b * hh2_c) max g2_c      (ONE fused DVE tensor_scalar)
    numer.T[o,i] & rowsum[i] via PE:  lhsT = [Wh | 1] (65 cols), rhs = u
    out[i,:] = elu(numer[i,:]/rowsum[i]),
    elu(x) = max(x, min(exp(x)-1, 0))   (exact identity)

Sharding: batch dim 16 -> 8 cores x 2 graphs (pure data parallel).
Node ids use the (p c) permutation -- partition p holds rows 8p..8p+7 --
so every h/out DMA run is 1KB+ contiguous per partition; softmax sums
are permutation-invariant and the store applies the inverse permutation.
Emission is phase-interleaved (A0 A1 B0 B1 C0 C1) so the two graphs
pipeline across engines.
"""

import numpy as np

import concourse.bass as bass
import concourse.mybir as mybir
import concourse.tile as tile
from concourse import bacc

F32 = mybir.dt.float32
BF16 = mybir.dt.bfloat16
AF = mybir.ActivationFunctionType
OP = mybir.AluOpType

B_PER_CORE = 2
N = 1024
F_IN = 128
F_OUT = 64
C = N // 128  # 8 chunks of 128 rows
NEG_SLOPE = 0.01
WARM = 8

LAST_PERF = {}


def build_bass():
    nc = bacc.Bacc("TRN2", target_bir_lowering=False, debug=False)

    h_d = nc.dram_tensor("h", [B_PER_CORE, N, F_IN], F32, kind="ExternalInput")
    w_d = nc.dram_tensor("W", [B_PER_CORE, F_IN, F_OUT], F32, kind="ExternalInput")
    a_d = nc.dram_tensor("a", [B_PER_CORE, 1, 2 * F_OUT, 1], F32, kind="ExternalInput")
    o_d = nc.dram_tensor("out", [B_PER_CORE, N, F_OUT], F32, kind="ExternalOutput")

    with tile.TileContext(nc) as tc:
        with (
            tc.tile_pool(name="singles", bufs=1) as singles,
            tc.tile_pool(name="hin", bufs=2) as hin_pool,
            tc.tile_pool(name="ht", bufs=2) as ht_pool,
            tc.tile_pool(name="small", bufs=2) as small_pool,
            tc.tile_pool(name="bcast", bufs=2) as bcast_pool,
            tc.tile_pool(name="v", bufs=16) as v_pool,
            tc.tile_pool(name="tail", bufs=2) as tail_pool,
            tc.tile_pool(name="ps", bufs=7, space="PSUM") as ps,
            tc.tile_pool(name="pswarm", bufs=1, space="PSUM") as pswarm,
        ):
            # identity built on-chip (no DMA): 1 where col==row else 0
            ident = singles.tile([128, 128], F32)
            nc.gpsimd.memset(ident[:], 1.0)
            nc.gpsimd.affine_select(
                ident[:], ident[:], pattern=[[1, 128]],
                compare_op=OP.is_equal, fill=0.0,
                base=0, channel_multiplier=-1,
            )
            zeros = singles.tile([128, 128], F32)
            nc.vector.memset(zeros[:], 0.0)

            warm_ps = pswarm.tile([128, 128], F32, tag="warm")

            st = [dict() for _ in range(B_PER_CORE)]

            def stage_load(b):
                """W/a DMAs first on each ring so the wt/wa preamble PE work
                can start as soon as the rings spin up."""
                s = st[b]
                dmaw = nc.scalar if b == 0 else nc.sync
                wext = small_pool.tile([128, F_OUT + 1], F32, tag="wext")
                dmaw.dma_start(out=wext[:, 0:F_OUT], in_=w_d[b])
                apair = small_pool.tile([F_OUT, 2], F32, tag="apair")
                dmaw.dma_start(
                    out=apair[:],
                    in_=a_d[b, 0, :, 0].rearrange("(two o) -> o two", two=2),
                )
                s.update(wext=wext, apair=apair)

            def stage_h(b):
                """One 512B-per-partition piece per chunk, alternating across
                the two DMA rings (fine pieces let the first h-transposes
                start while the rest of h streams in)."""
                s = st[b]
                h_sb = hin_pool.tile([128, C * F_IN], F32, tag="h")
                h_src = h_d[b].rearrange("(p c) f -> p (c f)", c=C)
                w = F_IN
                for q in range(8):
                    eng = (nc.sync, nc.scalar)[q % 2]
                    eng.dma_start(
                        out=h_sb[:, q * w : (q + 1) * w],
                        in_=h_src[:, q * w : (q + 1) * w],
                    )
                s["h_sb"] = h_sb

            def stage_wt(b):
                """W.T, wa12 = W @ [a1|a2], wa1 broadcast, bf16 W-ext.
                Runs in the preamble DMA window (before the h transposes)."""
                s = st[b]
                wext = s["wext"]
                wt_ps = ps.tile([F_OUT, 128], F32, tag="ps")
                nc.tensor.transpose(wt_ps[:], wext[:, 0:F_OUT], ident[:])
                wt_sb = small_pool.tile([F_OUT, 128], F32, tag="wt")
                nc.vector.tensor_copy(wt_sb[:], wt_ps[:])

                wa_ps = ps.tile([128, 2], F32, tag="ps")
                nc.tensor.matmul(wa_ps[:], wt_sb[:], s["apair"][:])
                wa_sb = small_pool.tile([128, 2], F32, tag="wa")
                nc.vector.tensor_copy(wa_sb[:], wa_ps[:])
                # wa2 becomes column 64 of the Wh matmul rhs -> f2 per chunk
                nc.vector.tensor_copy(wext[:, F_OUT : F_OUT + 1], wa_sb[:, 1:2])
                wextb = small_pool.tile([128, F_OUT + 1], BF16, tag="wextb")
                nc.vector.tensor_copy(wextb[:], wext[:])
                # broadcast wa1 along free dim -> lhsT for the f1-broadcast mm
                wa1b = small_pool.tile([128, 128], BF16, tag="wa1b")
                nc.vector.tensor_scalar(
                    wa1b[:], zeros[:], wa_sb[:, 0:1], None, op0=OP.add
                )
                s.update(wextb=wextb, wa1b=wa1b)

            def stage_a(b):
                """h transposes (bf16 ht), Wh+f2, small exps (g2, hh2),
                whb = [Wh|1], f1 broadcast, r1b = exp(-0.99 f1b)."""
                s = st[b]
                # transpose h -> ht [f, n] (bf16: feeds only bf16 matmuls)
                ht_sb = ht_pool.tile([128, N], BF16, tag="ht")
                for half in range(2):
                    pt = ps.tile([128, 4, 128], F32, tag="ps")
                    for q in range(4):
                        c = half * 4 + q
                        nc.tensor.transpose(
                            pt[:, q, :],
                            s["h_sb"][:, c * F_IN : (c + 1) * F_IN],
                            ident[:],
                        )
                    dst = ht_sb[:, half * 512 : (half + 1) * 512]
                    if half == 0:
                        nc.scalar.copy(dst, pt[:])
                    else:
                        nc.vector.tensor_copy(dst, pt[:])

                # Wh chunks (+f2 col): out[:, c, 0:64] = Wh_c, [:, c, 64] = f2_c
                pwh = []
                for half in range(2):
                    p = ps.tile([128, 4, F_OUT + 1], F32, tag="ps")
                    pwh.append(p)
                    for q in range(4):
                        c = half * 4 + q
                        nc.tensor.matmul(
                            p[:, q, :],
                            ht_sb[:, c * 128 : (c + 1) * 128],
                            s["wextb"][:],
                        )

                # B-phase per-partition scalars: g2 = exp(f2), hh2 = exp(.01 f2)
                g2 = small_pool.tile([128, C], F32, tag="g2")
                hh2 = small_pool.tile([128, C], F32, tag="hh2")
                for half in range(2):
                    sl = slice(half * 4, (half + 1) * 4)
                    nc.scalar.activation(g2[:, sl], pwh[half][:, :, F_OUT], AF.Exp)
                for half in range(2):
                    sl = slice(half * 4, (half + 1) * 4)
                    nc.scalar.activation(
                        hh2[:, sl], pwh[half][:, :, F_OUT], AF.Exp, scale=NEG_SLOPE
                    )

                # f1 broadcast to all 128 partitions via PE (bf16 fast path)
                pf1b = []
                for half in range(2):
                    p = ps.tile([128, 512], F32, tag="ps")
                    pf1b.append(p)
                    nc.tensor.matmul(
                        p[:], s["wa1b"][:], ht_sb[:, half * 512 : (half + 1) * 512]
                    )

                # r1b = exp(-0.99 f1b)  [128, 1024] bf16 (B-phase in0)
                r1b = bcast_pool.tile([128, N], BF16, tag="r1b")
                for half in range(2):
                    sl = slice(half * 512, (half + 1) * 512)
                    nc.scalar.activation(
                        r1b[:, sl], pf1b[half][:], AF.Exp,
                        scale=-(1.0 - NEG_SLOPE),
                    )

                # whb = [Wh | 1 | 1]: 66 cols so the per-chunk 64-col bf16
                # writes stay 4-byte aligned; col 64 is the rowsum ones-column.
                whb = small_pool.tile([128, C, F_OUT + 2], BF16, tag="whb")
                nc.vector.memset(whb[:], 1.0)
                for half in range(2):
                    nc.vector.tensor_copy(
                        whb[:, half * 4 : (half + 1) * 4, 0:F_OUT],
                        pwh[half][:, :, 0:F_OUT],
                    )
                s.update(ht=ht_sb, g2=g2, hh2=hh2, whb=whb, r1b=r1b)

            def stage_b(b):
                """Per chunk: u_c = (r1b * hh2_c) max g2_c in ONE fused DVE op,
                then the accumulating final matmuls (numer.T | rowsum)."""
                s = st[b]
                u_tiles = []
                for c in range(C):
                    u = v_pool.tile([128, N], BF16, tag="u")
                    u_tiles.append(u)
                    nc.vector.tensor_scalar(
                        u[:], s["r1b"][:],
                        s["hh2"][:, c : c + 1], s["g2"][:, c : c + 1],
                        op0=OP.mult, op1=OP.max,
                    )

                phpT = []
                for half in range(2):
                    p = ps.tile([F_OUT + 1, 512], F32, tag="ps")
                    phpT.append(p)
                    for c in range(C):
                        nc.tensor.matmul(
                            p[:],
                            s["whb"][:, c, 0 : F_OUT + 1],
                            u_tiles[c][:, half * 512 : (half + 1) * 512],
                            start=(c == 0),
                            stop=(c == C - 1),
                        )
                s["phpT"] = phpT

            def stage_c(b):
                """Transpose numer.T back, normalize, ELU, store."""
                s = st[b]
                hpT_sb = tail_pool.tile([F_OUT + 1, N], F32, tag="hpT")
                for half in range(2):
                    dst = hpT_sb[:, half * 512 : (half + 1) * 512]
                    if half == 0:
                        nc.scalar.copy(dst, s["phpT"][half][:])
                    else:
                        nc.vector.tensor_copy(dst, s["phpT"][half][:])
                php = []
                for half in range(2):
                    p = ps.tile([128, 4, F_OUT + 1], F32, tag="ps")
                    php.append(p)
                    for q in range(4):
                        c = half * 4 + q
                        nc.tensor.transpose(
                            p[:, q, :],
                            hpT_sb[:, c * 128 : (c + 1) * 128],
                            ident[: F_OUT + 1, : F_OUT + 1],
                        )

                rz = small_pool.tile([128, C], F32, tag="rz")
                for half in range(2):
                    sl = slice(half * 4, (half + 1) * 4)
                    nc.vector.reciprocal(rz[:, sl], php[half][:, :, F_OUT])
                # normalize all 4 chunks of a half in ONE tensor_tensor with a
                # 0-stride broadcast of rz along the feature dim (the 8 small
                # per-chunk ops were fixed-overhead dominated)
                hp = tail_pool.tile([128, C, F_OUT], F32, tag="hp")
                for half in range(2):
                    sl = slice(half * 4, (half + 1) * 4)
                    rz3 = rz[:, sl].rearrange("p (c one) -> p c one", one=1)
                    b_in0, b_rz = bass.broadcast_tensor_aps(
                        php[half][:, :, 0:F_OUT], rz3
                    )
                    nc.vector.tensor_tensor(hp[:, sl], b_in0, b_rz, op=OP.mult)
                # elu(x) = max(x, min(exp(x)-1, 0)); per-half so the store DMA
                # of half 0 overlaps the tail compute of half 1.
                te = tail_pool.tile([128, C, F_OUT], F32, tag="te")
                sm = tail_pool.tile([128, C, F_OUT], F32, tag="sm")
                osb = tail_pool.tile([128, C, F_OUT], F32, tag="osb")
                o_dst = o_d[b].rearrange("(p c) o -> p (c o)", c=C)
                osb_flat = osb[:].rearrange("p c o -> p (c o)")
                w = 4 * F_OUT
                for half in range(2):
                    sl = slice(half * 4, (half + 1) * 4)
                    nc.scalar.activation(te[:, sl], hp[:, sl], AF.Exp)
                    nc.vector.tensor_scalar(
                        sm[:, sl], te[:, sl], 1.0, 0.0,
                        op0=OP.subtract, op1=OP.min,
                    )
                    nc.vector.tensor_tensor(
                        osb[:, sl], hp[:, sl], sm[:, sl], op=OP.max
                    )
                    eng = (nc.sync, nc.scalar) if b == 0 else (nc.scalar, nc.sync)
                    eng[half].dma_start(
                        out=o_dst[:, half * w : (half + 1) * w],
                        in_=osb_flat[:, half * w : (half + 1) * w],
                    )

            # Ring issue order == emission order: W/a first (tiny), then the
            # h pieces; consumers are emitted before later transfers on the
            # same ring so their coarse ring-sem waits stay tight.
            stage_load(0)
            stage_load(1)
            stage_wt(0)
            stage_wt(1)
            stage_h(0)
            stage_h(1)
            # PE warm-up: junk matmuls in the DMA window so the HAM clock gate
            # ramps before the real PE work arrives.
            for _ in range(WARM):
                nc.tensor.matmul(warm_ps[:], zeros[:], zeros[:])
            stage_a(0)
            stage_a(1)
            # keep PE busy across the A->B seam (waiting on r1b/u) so the
            # HAM clock gate never ramps back down mid-kernel
            for _ in range(6):
                nc.tensor.matmul(warm_ps[:], zeros[:], zeros[:])
            stage_b(0)
            stage_b(1)
            stage_c(0)
            stage_c(1)

    nc.compile()
    return nc


def kernel(h: np.ndarray, W: np.ndarray, a: np.ndarray, _trace: bool = False):
    from concourse.bass_utils import run_bass_kernel_spmd

    n_cores = 8
    nc = build_bass()
    in_maps = []
    for i in range(n_cores):
        sl = slice(i * B_PER_CORE, (i + 1) * B_PER_CORE)
        in_maps.append(
            {
                "h": np.ascontiguousarray(h[sl]),
                "W": np.ascontiguousarray(W[sl]),
                "a": np.ascontiguousarray(a[sl]),
            }
        )
    res = run_bass_kernel_spmd(
        nc, in_maps, core_ids=list(range(n_cores)), trace=_trace
    )
    LAST_PERF.clear()
    LAST_PERF.update(
        {
            "exec_time_ns": res.exec_time_ns,
            "mean_exec_time_ns": res.mean_exec_time_ns,
            "trace": res.instructions_and_trace[1]
            if res.instructions_and_trace
            else None,
        }
    )
    return np.concatenate([r["out"] for r in res.results], axis=0)
